# revision 1
# baseline (speedup 1.0000x reference)
"""Causal multi-head attention block (16 heads, dim 1024) on 8 TRN2 NeuronCores.

Sharding: tensor-parallel over heads - core c computes heads {2c, 2c+1}:
  q/k/v projections with the 128-column weight slices, causal attention,
  and a partial output projection with the matching 128 Wout rows.
Host sums the 8 partial outputs and adds the bias.

Design notes:
  * The host supplies x PRE-TRANSPOSED (xT [dim, b*n], bf16), so the kernel
    contains ZERO PE-mode transposes (each measured ~275ns; the transpose-
    heavy variant spent ~140us/core on them).
  * q/k are computed feature-major (qT/kT [feat, tok]) for the score
    matmuls: lhsT = W slice, rhs = xT chunk (N=512 streams). v is computed
    TOKEN-major directly: lhsT = xT tile, rhs = Wv slice -> v[tok, feat],
    copied into vaug ([128 j, 65] per j-tile per head: 64 v cols + a ones
    col that makes the AV matmul also produce softmax denominators).
  * scores TRANSPOSED: dotsT[j,i] = kT.T @ qT per (j-tile, head), K=64 ->
    the two heads go to PE row groups (0,0)/(64,0) and run CONCURRENTLY;
    both land in one [128, 2048] psum tile so a single ACT exp op covers a
    j-tile pair x both heads (bigger ACT ops amortize its ~300ns/op fixed
    cost - ACT is within ~25% of being the bottleneck engine).
    Diagonal tiles are zeroed above the diagonal with gpsimd affine_select.
  * Software pipelining at emission order (engines execute their streams
    IN ORDER): the AV matmuls of a pair are emitted one pair LATE so the
    PE is never parked waiting on that pair's exp; the output projection
    of chunk c is deferred into chunk c+2's pair loop so the normalize
    chain (SBUF-staged reciprocal_approx_fast + partition_broadcast + stt;
    exact DVE reciprocal costs 3.3us and stalled everything) is never on
    the PE's critical path; phase12(b+1) emission is interleaved with
    phase34(b) at ~1:1 step granularity to fill ACT-bound stretches.
  * All DMA stays on nc.sync: issuing DMA from scalar/gpsimd blocks those
    engines' in-order queues on the DMA's dependencies (measured -10%).
"""
import numpy as np
import ml_dtypes
from contextlib import ExitStack, nullcontext

import concourse.bacc as bacc
import concourse.mybir as mybir
import concourse.tile as tile
import concourse.bass_utils as bass_utils

F32 = mybir.dt.float32
BF16 = mybir.dt.bfloat16
FP16 = mybir.dt.float16

B = 4            # batches
T = 2048         # tokens per batch
DIM = 1024
NT = T // 128    # token tiles per batch (16)
KT = DIM // 128  # contraction tiles (8)
NCHUNK = T // 512  # 512-col i-chunks per batch (4)
SCALE = DIM ** -0.5  # 1/32 - NOTE: full dim, not head dim (matches reference)

TRACE = False
LAST_EXEC_NS = None
LAST_TRACE = None
LAST_PROFILE = None
_CACHED = {}


def build_kernel(nbatches=None, interleave=True):
    NB = nbatches if nbatches is not None else B

    nc = bacc.Bacc("TRN2", target_bir_lowering=False, debug=False, num_devices=8)

    xT_d = nc.dram_tensor("xt", [DIM, B * T], BF16, kind="ExternalInput").ap()
    wq_d = nc.dram_tensor("wq", [DIM, 128], BF16, kind="ExternalInput").ap()
    wk_d = nc.dram_tensor("wk", [DIM, 128], BF16, kind="ExternalInput").ap()
    wv_d = nc.dram_tensor("wv", [DIM, 128], BF16, kind="ExternalInput").ap()
    wo_d = nc.dram_tensor("wo", [128, DIM], BF16, kind="ExternalInput").ap()
    out_d = nc.dram_tensor("out", [B * T, DIM], FP16, kind="ExternalOutput").ap()

    with tile.TileContext(nc) as tc, ExitStack() as ctx:
        cp = ctx.enter_context(tc.tile_pool(name="const", bufs=1))
        xT_p = ctx.enter_context(tc.tile_pool(name="xT", bufs=2))
        qT_p = ctx.enter_context(tc.tile_pool(name="qT", bufs=2))
        kT_p = ctx.enter_context(tc.tile_pool(name="kT", bufs=2))
        vaug_p = ctx.enter_context(tc.tile_pool(name="vaug", bufs=2))
        attnT_p = ctx.enter_context(tc.tile_pool(name="attnT", bufs=4))
        recip_p = ctx.enter_context(tc.tile_pool(name="recip", bufs=4))
        rbc_p = ctx.enter_context(tc.tile_pool(name="rbc", bufs=2))
        outT_p = ctx.enter_context(tc.tile_pool(name="outT", bufs=2))
        osb_p = ctx.enter_context(tc.tile_pool(name="osb", bufs=3))
        mm_ps = ctx.enter_context(tc.tile_pool(name="mmps", bufs=2, space="PSUM"))
        dots_ps = ctx.enter_context(tc.tile_pool(name="dotsps", bufs=1, space="PSUM"))
        av_ps_p = ctx.enter_context(tc.tile_pool(name="avps", bufs=2, space="PSUM"))

        # ---- constants ----
        ones32 = cp.tile([128, NT], F32, tag="ones32")
        nc.gpsimd.memset(ones32[:], 1.0)

        # ---- weights ----
        wq_sb = cp.tile([128, KT * 128], BF16, tag="wq")
        wk_sb = cp.tile([128, KT * 128], BF16, tag="wk")
        wv_sb = cp.tile([128, KT * 128], BF16, tag="wv")
        wo_sb = cp.tile([128, DIM], BF16, tag="wo")
        for w_sb, w_d in ((wq_sb, wq_d), (wk_sb, wk_d), (wv_sb, wv_d)):
            nc.sync.dma_start(w_sb[:].rearrange("p (kt m) -> p kt m", kt=KT),
                              w_d.rearrange("(kt p) m -> p kt m", p=128))
        nc.sync.dma_start(wo_sb[:], wo_d)

        state = {}  # per-batch qT/kT/vaug handles

        def phase12_steps(b):
            """xT DMA + q/k (feat-major) + v (token-major) for batch b."""
            t0 = b * T
            xT = xT_p.tile([128, KT * T], BF16, tag="xT", name="xT")
            xTv = xT[:].rearrange("p (kt t) -> p kt t", kt=KT)
            xsrc = xT_d.rearrange("(kt p) t -> p kt t", p=128)

            def dma_slice(ch):
                nc.sync.dma_start(xTv[:, :, ch * 512:(ch + 1) * 512],
                                  xsrc[:, :, t0 + ch * 512: t0 + (ch + 1) * 512])

            # keep the DMA 2 token-slices ahead of the projections
            dma_slice(0)
            dma_slice(1)
            yield
            qT = qT_p.tile([128, T], BF16, tag="qT", name="qT")
            kTt = kT_p.tile([128, T], BF16, tag="kT", name="kT")
            vaug = vaug_p.tile([128, NT * 130], BF16, tag="vaug", name="vaug")
            vv = vaug[:].rearrange("p (jt c) -> p jt c", c=130)
            for ch in range(NCHUNK):
                if ch + 2 < NCHUNK:
                    dma_slice(ch + 2)
                # q and k projections for this 512-token chunk; yield
                # mid-accumulation so phase12 PE work spreads evenly across
                # the interleaved phase34 steps
                for w_sb, dest in ((wq_sb, qT), (wk_sb, kTt)):
                    pp = mm_ps.tile([128, 512], F32, tag="mm", name="pp")
                    for kt in range(KT):
                        nc.tensor.matmul(
                            pp[:], w_sb[:, kt * 128:(kt + 1) * 128],
                            xTv[:, kt, ch * 512:(ch + 1) * 512],
                            start=(kt == 0), stop=(kt == KT - 1))
                        if kt == 3:
                            yield
                    nc.vector.tensor_copy(dest[:, ch * 512:(ch + 1) * 512], pp[:])
                    yield
                # v for the same 4 token-tiles, token-major
                vp = mm_ps.tile([128, 512], F32, tag="mm", name="vp")
                for j in range(4):
                    tt = 4 * ch + j
                    for kt in range(KT):
                        nc.tensor.matmul(
                            vp[:, j * 128:(j + 1) * 128],
                            xTv[:, kt, tt * 128:(tt + 1) * 128],
                            wv_sb[:, kt * 128:(kt + 1) * 128],
                            start=(kt == 0), stop=(kt == KT - 1))
                    if j == 1:
                        yield
                src = vp[:].rearrange("p (j c) -> p j c", j=4)
                nc.vector.tensor_copy(vv[:, 4 * ch:4 * ch + 4, 0:64], src[:, :, 0:64])
                nc.vector.tensor_copy(vv[:, 4 * ch:4 * ch + 4, 65:129], src[:, :, 64:128])
                yield
            nc.vector.tensor_copy(
                vaug[:].rearrange("p (u c) -> p u c", c=130)[:, :, 64:65],
                ones32[:].rearrange("p (u o) -> p u o", o=1))
            nc.vector.tensor_copy(
                vaug[:].rearrange("p (u c) -> p u c", c=130)[:, :, 129:130],
                ones32[:].rearrange("p (u o) -> p u o", o=1))
            state[b] = (qT, kTt, vaug)

        def phase34_steps(b):
            """Attention + deferred output projection for batch b.

            The oproj of chunk c is emitted interleaved into chunk c+1's
            pair loop, so the PE never waits on the normalize chain."""
            t0 = b * T
            qT, kTt, vaug = state.pop(b)
            outT = outT_p.tile([128, T], BF16, tag="outT", name="outT")
            deferred = []  # token-tiles whose oproj is pending

            def emit_oproj(tt):
                osb = osb_p.tile([128, DIM], FP16, tag="osb", name="osb")
                for half in (0, 1):
                    po = mm_ps.tile([128, 512], F32, tag="mm", name="po")
                    nc.tensor.matmul(po[:], outT[:, tt * 128:(tt + 1) * 128],
                                     wo_sb[:, half * 512:(half + 1) * 512],
                                     start=True, stop=True)
                    nc.vector.tensor_copy(osb[:, half * 512:(half + 1) * 512],
                                          po[:])
                nc.sync.dma_start(out_d[t0 + tt * 128: t0 + (tt + 1) * 128, :],
                                  osb[:])

            for c in range(NCHUNK):
                njt = 4 * (c + 1)
                avp = {h: av_ps_p.tile([65, 512], F32, tag="av", name=f"avp{h}")
                       for h in (0, 1)}

                def emit_av(pend, njt=njt, avp=avp):
                    jts, offs, at = pend
                    for h in (0, 1):
                        for j, jt in enumerate(jts):
                            off = offs[j]
                            nc.tensor.matmul(
                                avp[h][:, off:512],
                                vaug[:, jt * 130 + 65 * h: jt * 130 + 65 * h + 65],
                                at[:, h * 1024 + j * 512 + off: h * 1024 + (j + 1) * 512],
                                start=(jt == 0), stop=(jt == njt - 1))

                pend = None  # exp'd pair awaiting its AV (one-pair software pipeline)
                for jp in range(njt // 2):
                    jts = (2 * jp, 2 * jp + 1)
                    offs = [max(512 * c, jt * 128) - 512 * c for jt in jts]
                    # AV of the previous pair first: its exp finished during the
                    # last step, so the PE is never parked behind the ACT engine
                    if pend is not None:
                        emit_av(pend)
                    # one psum tile per pair: h0 cols [0:1024], h1 [1024:2048]
                    dps = dots_ps.tile([128, 2048], F32, tag="dots", name="dp")
                    # j outer, h inner: adjacent matmuls hit disjoint PE row
                    # groups (h0 rows 0-63, h1 rows 64-127) and run concurrently
                    for j, jt in enumerate(jts):
                        off = offs[j]
                        for h in (0, 1):
                            nc.tensor.matmul(
                                dps[:, h * 1024 + j * 512 + off: h * 1024 + (j + 1) * 512],
                                kTt[64 * h:64 * h + 64, jt * 128:(jt + 1) * 128],
                                qT[64 * h:64 * h + 64, 512 * c + off:512 * (c + 1)],
                                start=True, stop=True)
                    # single exp over both heads' scores
                    at = attnT_p.tile([128, 2048], BF16, tag="at", name="at")
                    nc.scalar.activation(at[:, offs[0]:2048], dps[:, offs[0]:2048],
                                         mybir.ActivationFunctionType.Exp,
                                         bias=0.0, scale=float(SCALE))
                    for h in (0, 1):
                        for j, jt in enumerate(jts):
                            if jt >= 4 * c:  # zero invalid (j > i) entries
                                base = h * 1024 + j * 512 + offs[j]
                                nc.gpsimd.affine_select(
                                    out=at[:, base: base + 128],
                                    in_=at[:, base: base + 128],
                                    compare_op=mybir.AluOpType.is_ge, fill=0.0,
                                    base=0, pattern=[[1, 128]], channel_multiplier=-1)
                    # lag the oproj ~2 chunks behind so the normalize chain
                    # (recip+bcast+stt on DVE/gpsimd) is long done by then
                    if len(deferred) > 4:
                        emit_oproj(deferred.pop(0))
                    pend = (jts, offs, at)
                    yield
                emit_av(pend)
                # normalize immediately (fast recip keeps this short); the
                # dependent oproj is deferred into the next chunk's pairs
                for h in (0, 1):
                    # stage the denominator row into SBUF: the custom-DVE
                    # approx reciprocal mis-reads PSUM operands
                    den = recip_p.tile([1, 512], F32, tag="den", name="den")
                    nc.vector.tensor_copy(den[:], avp[h][64:65, :])
                    rc = recip_p.tile([1, 512], F32, tag="recip", name="rc")
                    nc.vector.reciprocal_approx_fast(rc[:], den[:])
                    rb = rbc_p.tile([64, 512], F32, tag="rbc", name="rb")
                    nc.gpsimd.partition_broadcast(rb[:], rc[:])
                    nc.vector.scalar_tensor_tensor(
                        outT[64 * h:64 * h + 64, c * 512:(c + 1) * 512],
                        avp[h][0:64, :], 1.0, rb[:],
                        op0=mybir.AluOpType.mult, op1=mybir.AluOpType.mult)
                deferred.extend(range(4 * c, 4 * c + 4))
                yield
            while deferred:
                emit_oproj(deferred.pop(0))
                yield

        def drive(gens):
            """Round-robin the emission generators until all are exhausted."""
            gens = [g for g in gens if g is not None]
            while gens:
                nxt = []
                for g in gens:
                    try:
                        next(g)
                        nxt.append(g)
                    except StopIteration:
                        pass
                gens = nxt

        if interleave:
            for b in range(NB + 1):
                drive([phase12_steps(b) if b < NB else None,
                       phase34_steps(b - 1) if b >= 1 else None])
        else:
            for b in range(NB):
                drive([phase12_steps(b)])
                drive([phase34_steps(b)])

    nc.compile()
    return nc


def kernel(x, Wq, Wkv, Wout, bout):
    """Full inputs -> full output. Shards across 8 NeuronCores internally."""
    global LAST_EXEC_NS, LAST_TRACE
    if "nc" not in _CACHED:
        _CACHED["nc"] = build_kernel()
    nc = _CACHED["nc"]

    hdt = ml_dtypes.bfloat16
    xf = np.asarray(x, dtype=np.float32).reshape(B * T, DIM)
    xT = np.ascontiguousarray(xf.T).astype(hdt)  # [DIM, B*T]
    Wq = np.asarray(Wq, dtype=np.float32).astype(hdt)
    Wkv = np.asarray(Wkv, dtype=np.float32).astype(hdt)
    Wout = np.asarray(Wout, dtype=np.float32).astype(hdt)
    bout = np.asarray(bout, dtype=np.float32)

    in_maps = []
    for c in range(8):
        s = slice(128 * c, 128 * (c + 1))
        in_maps.append({
            "xt": xT,
            "wq": np.ascontiguousarray(Wq[:, s]),
            "wk": np.ascontiguousarray(Wkv[:, :DIM][:, s]),
            "wv": np.ascontiguousarray(Wkv[:, DIM:][:, s]),
            "wo": np.ascontiguousarray(Wout[s, :]),
        })

    res = bass_utils.run_bass_kernel_spmd(nc, in_maps, core_ids=list(range(8)),
                                          trace=TRACE)
    if TRACE:
        LAST_EXEC_NS = res.exec_time_ns
        LAST_TRACE = res.instructions_and_trace
        globals()["LAST_PROFILE"] = getattr(res, "profile_json", None)
    acc = res.results[0]["out"].astype(np.float64)
    for c in range(1, 8):
        acc += res.results[c]["out"]
    out = (acc + bout.astype(np.float64)).astype(np.float32)
    return out.reshape(B, T, DIM)



# revision 2
# speedup vs baseline: 1.0939x; 1.0939x over previous
"""Causal multi-head attention block (16 heads, dim 1024) on 8 TRN2 NeuronCores.

Sharding: tensor-parallel over heads - core c computes heads {2c, 2c+1}:
  q/k/v projections with the 128-column weight slices, causal attention,
  and a partial output projection with the matching 128 Wout rows.
Host sums the 8 partial outputs and adds the bias.

Design notes (v2 - per-chunk software pipeline):
  * The host supplies x PRE-TRANSPOSED and chunk-blocked
    (xT [128, B*NCHUNK, KT, 512] bf16) so each per-chunk DMA reads 8KB
    contiguous per partition (the flat [dim, b*n] layout produced ~600B
    packets and 130us of DMA active time).
  * q/k are computed feature-major (qT/kT [feat, tok]) for the score
    matmuls: lhsT = W slice, rhs = xT chunk (N=512 streams). v is computed
    TOKEN-major directly -> vaug ([128 j, 65] per j-tile per head: 64 v
    cols + a ones col that makes the AV matmul also produce softmax
    denominators).
  * scores TRANSPOSED: dotsT[j,i] = kT.T @ qT per (j-tile, head), K=64 ->
    the two heads go to PE row groups (0,0)/(64,0) and run CONCURRENTLY;
    both land in one [128, 2048] psum tile so a single ACT exp op covers a
    j-tile pair x both heads. Diagonal tiles are zeroed above the diagonal
    with gpsimd affine_select.
  * Pipeline granularity is a (batch, 512-token chunk) unit: attention for
    unit u runs interleaved (via generator round-robin) with projections
    for unit u+1, so the PE always has independent matmul work queued
    while ACT runs the 1.9us exp of a score pair. This kills the 26us
    startup bubble and the ACT-bound tail the batch-level pipeline had
    (the PE also drops out of its 2.4GHz p-state after ~any stall, so
    gaps cost ~2x their length).
  * Within a pair iteration the emission order is scores(p) -> exp(p) ->
    AV(p-1) -> deferred oproj: exp(p) only waits on the 0.2us scores
    matmul, so consecutive exps chain back-to-back on ACT while the PE
    covers AV + oproj + projection filler under them.
  * The output projection of chunk c is deferred into chunk c+1's pair
    loop so the normalize chain (SBUF-staged reciprocal_approx_fast +
    partition_broadcast + stt) is never on the PE's critical path.
  * All DMA stays on nc.sync: issuing DMA from scalar/gpsimd blocks those
    engines' in-order queues on the DMA's dependencies (measured -10%).
"""
import numpy as np
import ml_dtypes
from contextlib import ExitStack

import concourse.bacc as bacc
import concourse.mybir as mybir
import concourse.tile as tile
import concourse.bass_utils as bass_utils

F32 = mybir.dt.float32
BF16 = mybir.dt.bfloat16
FP16 = mybir.dt.float16

B = 4            # batches
T = 2048         # tokens per batch
DIM = 1024
NT = T // 128    # token tiles per batch (16)
KT = DIM // 128  # contraction tiles (8)
NCHUNK = T // 512  # 512-col i-chunks per batch (4)
SCALE = DIM ** -0.5  # 1/32 - NOTE: full dim, not head dim (matches reference)

TRACE = False
LAST_EXEC_NS = None
LAST_TRACE = None
LAST_PROFILE = None
_CACHED = {}


def build_kernel():
    nc = bacc.Bacc("TRN2", target_bir_lowering=False, debug=False, num_devices=8)

    xT_d = nc.dram_tensor("xt", [128, B * NCHUNK * KT * 512], BF16,
                          kind="ExternalInput").ap()
    wq_d = nc.dram_tensor("wq", [DIM, 128], BF16, kind="ExternalInput").ap()
    wk_d = nc.dram_tensor("wk", [DIM, 128], BF16, kind="ExternalInput").ap()
    wv_d = nc.dram_tensor("wv", [DIM, 128], BF16, kind="ExternalInput").ap()
    wo_d = nc.dram_tensor("wo", [128, DIM], BF16, kind="ExternalInput").ap()
    out_d = nc.dram_tensor("out", [B * T, DIM], FP16, kind="ExternalOutput").ap()

    xsrc = xT_d.rearrange("p (u kt t) -> p u kt t", kt=KT, t=512)

    with tile.TileContext(nc) as tc, ExitStack() as ctx:
        cp = ctx.enter_context(tc.tile_pool(name="const", bufs=1))
        xT_p = ctx.enter_context(tc.tile_pool(name="xT", bufs=2))
        qT_p = ctx.enter_context(tc.tile_pool(name="qT", bufs=2))
        kT_p = ctx.enter_context(tc.tile_pool(name="kT", bufs=2))
        vaug_p = ctx.enter_context(tc.tile_pool(name="vaug", bufs=2))
        attnT_p = ctx.enter_context(tc.tile_pool(name="attnT", bufs=4))
        recip_p = ctx.enter_context(tc.tile_pool(name="recip", bufs=4))
        rbc_p = ctx.enter_context(tc.tile_pool(name="rbc", bufs=2))
        outT_p = ctx.enter_context(tc.tile_pool(name="outT", bufs=2))
        osb_p = ctx.enter_context(tc.tile_pool(name="osb", bufs=3))
        mm_ps = ctx.enter_context(tc.tile_pool(name="mmps", bufs=2, space="PSUM"))
        dots_ps = ctx.enter_context(tc.tile_pool(name="dotsps", bufs=1, space="PSUM"))
        av_ps_p = ctx.enter_context(tc.tile_pool(name="avps", bufs=2, space="PSUM"))

        # ---- weights (wq/wk first: the first q/k projections only wait on
        # them + the first x slice; wv/wo are DMA'd after the first x slices
        # via post_dma so the pipeline starts ~4us earlier) ----
        wq_sb = cp.tile([128, KT * 128], BF16, tag="wq")
        wk_sb = cp.tile([128, KT * 128], BF16, tag="wk")
        wv_sb = cp.tile([128, KT * 128], BF16, tag="wv")
        wo_sb = cp.tile([128, DIM], BF16, tag="wo")
        for w_sb, w_d in ((wq_sb, wq_d), (wk_sb, wk_d)):
            nc.sync.dma_start(w_sb[:].rearrange("p (kt m) -> p kt m", kt=KT),
                              w_d.rearrange("(kt p) m -> p kt m", p=128))

        def dma_wv_wo():
            nc.sync.dma_start(wv_sb[:].rearrange("p (kt m) -> p kt m", kt=KT),
                              wv_d.rearrange("(kt p) m -> p kt m", p=128))
            nc.sync.dma_start(wo_sb[:], wo_d)

        state = {}     # b -> (qT, kT, vaug, outT, xTv)
        deferred = []  # (outT, t0, tt) output-projection backlog

        def emit_oproj(outT, t0, tt):
            osb = osb_p.tile([128, DIM], FP16, tag="osb", name="osb")
            for half in (0, 1):
                po = mm_ps.tile([128, 512], F32, tag="mm", name="po")
                nc.tensor.matmul(po[:], outT[:, tt * 128:(tt + 1) * 128],
                                 wo_sb[:, half * 512:(half + 1) * 512],
                                 start=True, stop=True)
                nc.vector.tensor_copy(osb[:, half * 512:(half + 1) * 512],
                                      po[:])
            nc.sync.dma_start(out_d[t0 + tt * 128: t0 + (tt + 1) * 128, :],
                              osb[:])

        def p12_chunk_steps(b, ch, post_dma=None):
            """xT DMA + q/k (feat-major) + v (token-major) for chunk ch of
            batch b."""
            if ch == 0:
                xT = xT_p.tile([128, NCHUNK * KT * 512], BF16, tag="xT",
                               name="xT")
                xTv = xT[:].rearrange("p (c kt t) -> p c kt t", c=NCHUNK, kt=KT)
                qT = qT_p.tile([128, T], BF16, tag="qT", name="qT")
                kTt = kT_p.tile([128, T], BF16, tag="kT", name="kT")
                vaug = vaug_p.tile([128, NT * 130], BF16, tag="vaug", name="vaug")
                outT = outT_p.tile([128, T], BF16, tag="outT", name="outT")
                state[b] = (qT, kTt, vaug, outT, xTv)
            qT, kTt, vaug, outT, xTv = state[b]

            def dma_slice(c2):
                nc.sync.dma_start(xTv[:, c2, :, :], xsrc[:, b * NCHUNK + c2, :, :])

            # keep the DMA one chunk-slot ahead of the projections
            if ch == 0:
                dma_slice(0)
                dma_slice(1)
                if post_dma is not None:
                    post_dma()
            elif ch + 1 < NCHUNK:
                dma_slice(ch + 1)
            yield
            # q and k projections for this 512-token chunk; yield
            # mid-accumulation so the PE work spreads across the
            # interleaved attention pair steps
            for w_sb, dest in ((wq_sb, qT), (wk_sb, kTt)):
                pp = mm_ps.tile([128, 512], F32, tag="mm", name="pp")
                for kt in range(KT):
                    nc.tensor.matmul(pp[:], w_sb[:, kt * 128:(kt + 1) * 128],
                                     xTv[:, ch, kt, :],
                                     start=(kt == 0), stop=(kt == KT - 1))
                    if kt == 3:
                        yield
                nc.vector.tensor_copy(dest[:, ch * 512:(ch + 1) * 512], pp[:])
                yield
            # v for the same 4 token-tiles, token-major
            vv = vaug[:].rearrange("p (jt c) -> p jt c", c=130)
            vp = mm_ps.tile([128, 512], F32, tag="mm", name="vp")
            for j in range(4):
                for kt in range(KT):
                    nc.tensor.matmul(
                        vp[:, j * 128:(j + 1) * 128],
                        xTv[:, ch, kt, j * 128:(j + 1) * 128],
                        wv_sb[:, kt * 128:(kt + 1) * 128],
                        start=(kt == 0), stop=(kt == KT - 1))
                if j == 1:
                    yield
            src = vp[:].rearrange("p (j c) -> p j c", j=4)
            nc.vector.tensor_copy(vv[:, 4 * ch:4 * ch + 4, 0:64], src[:, :, 0:64])
            nc.vector.tensor_copy(vv[:, 4 * ch:4 * ch + 4, 65:129], src[:, :, 64:128])
            nc.gpsimd.memset(vv[:, 4 * ch:4 * ch + 4, 64:65], 1.0)
            nc.gpsimd.memset(vv[:, 4 * ch:4 * ch + 4, 129:130], 1.0)
            yield

        def att_chunk_steps(b, c):
            """Attention chunk c of batch b + deferred output projections."""
            t0 = b * T
            qT, kTt, vaug, outT, _ = state[b]
            njt = 4 * (c + 1)
            avp = {h: av_ps_p.tile([65, 512], F32, tag="av", name=f"avp{h}")
                   for h in (0, 1)}

            def emit_av(pend):
                jts, offs, at = pend
                for h in (0, 1):
                    for j, jt in enumerate(jts):
                        off = offs[j]
                        nc.tensor.matmul(
                            avp[h][:, off:512],
                            vaug[:, jt * 130 + 65 * h: jt * 130 + 65 * h + 65],
                            at[:, h * 1024 + j * 512 + off: h * 1024 + (j + 1) * 512],
                            start=(jt == 0), stop=(jt == njt - 1))

            pend = None  # exp'd pair awaiting its AV (one-pair software pipeline)
            for jp in range(njt // 2):
                jts = (2 * jp, 2 * jp + 1)
                offs = [max(512 * c, jt * 128) - 512 * c for jt in jts]
                # one psum tile per pair: h0 cols [0:1024], h1 [1024:2048]
                dps = dots_ps.tile([128, 2048], F32, tag="dots", name="dp")
                # j outer, h inner: adjacent matmuls hit disjoint PE row
                # groups (h0 rows 0-63, h1 rows 64-127) and run concurrently
                for j, jt in enumerate(jts):
                    off = offs[j]
                    for h in (0, 1):
                        nc.tensor.matmul(
                            dps[:, h * 1024 + j * 512 + off: h * 1024 + (j + 1) * 512],
                            kTt[64 * h:64 * h + 64, jt * 128:(jt + 1) * 128],
                            qT[64 * h:64 * h + 64, 512 * c + off:512 * (c + 1)],
                            start=True, stop=True)
                # single exp over both heads' scores, emitted BEFORE the
                # previous pair's AV: it only waits on the 0.2us scores
                # matmuls, so ACT exps chain back-to-back while the PE runs
                # AV(p-1) + oproj + projection filler underneath
                at = attnT_p.tile([128, 2048], BF16, tag="at", name="at")
                nc.scalar.activation(at[:, offs[0]:2048], dps[:, offs[0]:2048],
                                     mybir.ActivationFunctionType.Exp,
                                     bias=0.0, scale=float(SCALE))
                for h in (0, 1):
                    for j, jt in enumerate(jts):
                        if jt >= 4 * c:  # zero invalid (j > i) entries
                            base = h * 1024 + j * 512 + offs[j]
                            nc.gpsimd.affine_select(
                                out=at[:, base: base + 128],
                                in_=at[:, base: base + 128],
                                compare_op=mybir.AluOpType.is_ge, fill=0.0,
                                base=0, pattern=[[1, 128]], channel_multiplier=-1)
                if pend is not None:
                    emit_av(pend)
                if deferred:
                    emit_oproj(*deferred.pop(0))
                pend = (jts, offs, at)
                yield
            emit_av(pend)
            # normalize immediately (fast recip keeps this short); the
            # dependent oproj is deferred into the next chunk's pairs
            for h in (0, 1):
                # stage the denominator row into SBUF: the custom-DVE
                # approx reciprocal mis-reads PSUM operands
                den = recip_p.tile([1, 512], F32, tag="den", name="den")
                nc.vector.tensor_copy(den[:], avp[h][64:65, :])
                rc = recip_p.tile([1, 512], F32, tag="recip", name="rc")
                nc.vector.reciprocal_approx_fast(rc[:], den[:])
                rb = rbc_p.tile([64, 512], F32, tag="rbc", name="rb")
                nc.gpsimd.partition_broadcast(rb[:], rc[:])
                nc.vector.scalar_tensor_tensor(
                    outT[64 * h:64 * h + 64, c * 512:(c + 1) * 512],
                    avp[h][0:64, :], 1.0, rb[:],
                    op0=mybir.AluOpType.mult, op1=mybir.AluOpType.mult)
            deferred.extend((outT, t0, tt) for tt in range(4 * c, 4 * c + 4))
            yield
            if c == NCHUNK - 1:
                # end of batch: flush the backlog (interleaved by drive()
                # with the next batch's first projection chunk)
                while deferred:
                    emit_oproj(*deferred.pop(0))
                    yield

        def drive(gens):
            """Round-robin the emission generators until all are exhausted."""
            gens = [g for g in gens if g is not None]
            while gens:
                nxt = []
                for g in gens:
                    try:
                        next(g)
                        nxt.append(g)
                    except StopIteration:
                        pass
                gens = nxt

        units = [(b, ch) for b in range(B) for ch in range(NCHUNK)]
        prev = None
        for i, (b, ch) in enumerate(units):
            drive([att_chunk_steps(*prev) if prev is not None else None,
                   p12_chunk_steps(b, ch, post_dma=dma_wv_wo if i == 0 else None)])
            prev = (b, ch)
        drive([att_chunk_steps(*prev)])

    nc.compile()
    return nc


def kernel(x, Wq, Wkv, Wout, bout):
    """Full inputs -> full output. Shards across 8 NeuronCores internally."""
    global LAST_EXEC_NS, LAST_TRACE
    if "nc" not in _CACHED:
        _CACHED["nc"] = build_kernel()
    nc = _CACHED["nc"]

    hdt = ml_dtypes.bfloat16
    xf = np.asarray(x, dtype=np.float32).reshape(B, NCHUNK, 512, DIM)
    # [128, B, NCHUNK, KT, 512]: per-(chunk,partition) rows are 8KB
    # contiguous in DRAM so the per-chunk DMA moves full-size packets
    xT = np.ascontiguousarray(
        xf.transpose(3, 0, 1, 2)            # [DIM, B, NCHUNK, 512]
          .reshape(KT, 128, B, NCHUNK, 512)
          .transpose(1, 2, 3, 0, 4)).astype(hdt)
    xT = xT.reshape(128, B * NCHUNK * KT * 512)
    Wq = np.asarray(Wq, dtype=np.float32).astype(hdt)
    Wkv = np.asarray(Wkv, dtype=np.float32).astype(hdt)
    Wout = np.asarray(Wout, dtype=np.float32).astype(hdt)
    bout = np.asarray(bout, dtype=np.float32)

    in_maps = []
    for c in range(8):
        s = slice(128 * c, 128 * (c + 1))
        in_maps.append({
            "xt": xT,
            "wq": np.ascontiguousarray(Wq[:, s]),
            "wk": np.ascontiguousarray(Wkv[:, :DIM][:, s]),
            "wv": np.ascontiguousarray(Wkv[:, DIM:][:, s]),
            "wo": np.ascontiguousarray(Wout[s, :]),
        })

    res = bass_utils.run_bass_kernel_spmd(nc, in_maps, core_ids=list(range(8)),
                                          trace=TRACE)
    if TRACE:
        LAST_EXEC_NS = res.exec_time_ns
        LAST_TRACE = res.instructions_and_trace
        globals()["LAST_PROFILE"] = getattr(res, "profile_json", None)
    acc = res.results[0]["out"].astype(np.float64)
    for c in range(1, 8):
        acc += res.results[c]["out"]
    out = (acc + bout.astype(np.float64)).astype(np.float32)
    return out.reshape(B, T, DIM)


# revision 16
# speedup vs baseline: 1.1174x; 1.0215x over previous
"""Causal multi-head attention block (16 heads, dim 1024) on 8 TRN2 NeuronCores.

Sharding: tensor-parallel over heads - core c computes heads {2c, 2c+1}:
  q/k/v projections with the 128-column weight slices, causal attention,
  and a partial output projection with the matching 128 Wout rows.
Host sums the 8 partial outputs and adds the bias.

Design notes (v2 - per-chunk software pipeline):
  * The host supplies x PRE-TRANSPOSED and chunk-blocked
    (xT [128, B*NCHUNK, KT, 512] bf16) so each per-chunk DMA reads 8KB
    contiguous per partition (the flat [dim, b*n] layout produced ~600B
    packets and 130us of DMA active time).
  * q/k are computed feature-major (qT/kT [feat, tok]) for the score
    matmuls: lhsT = W slice, rhs = xT chunk (N=512 streams). v is computed
    TOKEN-major directly -> vaug ([128 j, 65] per j-tile per head: 64 v
    cols + a ones col that makes the AV matmul also produce softmax
    denominators).
  * scores TRANSPOSED: dotsT[j,i] = kT.T @ qT per (j-tile, head), K=64 ->
    the two heads go to PE row groups (0,0)/(64,0) and run CONCURRENTLY;
    both land in one [128, 2048] psum tile so a single ACT exp op covers a
    j-tile pair x both heads. Diagonal tiles are zeroed above the diagonal
    with gpsimd affine_select.
  * Pipeline granularity is a (batch, 512-token chunk) unit: attention for
    unit u runs interleaved (via generator round-robin) with projections
    for unit u+1, so the PE always has independent matmul work queued
    while ACT runs the 1.9us exp of a score pair. This kills the 26us
    startup bubble and the ACT-bound tail the batch-level pipeline had
    (the PE also drops out of its 2.4GHz p-state after ~any stall, so
    gaps cost ~2x their length).
  * Within a pair iteration the emission order is scores(p) -> exp(p) ->
    AV(p-1) -> deferred oproj: exp(p) only waits on the 0.2us scores
    matmul, so consecutive exps chain back-to-back on ACT while the PE
    covers AV + oproj + projection filler under them.
  * The output projection of chunk c is deferred into chunk c+1's pair
    loop so the normalize chain (SBUF-staged reciprocal_approx_fast +
    partition_broadcast + stt) is never on the PE's critical path.
  * All DMA stays on nc.sync: issuing DMA from scalar/gpsimd blocks those
    engines' in-order queues on the DMA's dependencies (measured -10%).
"""
import numpy as np
import ml_dtypes
from contextlib import ExitStack

import concourse.bacc as bacc
import concourse.mybir as mybir
import concourse.tile as tile
import concourse.bass_utils as bass_utils

F32 = mybir.dt.float32
BF16 = mybir.dt.bfloat16
FP16 = mybir.dt.float16

B = 4            # batches
T = 2048         # tokens per batch
DIM = 1024
NT = T // 128    # token tiles per batch (16)
KT = DIM // 128  # contraction tiles (8)
NCHUNK = T // 512  # 512-col i-chunks per batch (4)
SCALE = DIM ** -0.5  # 1/32 - NOTE: full dim, not head dim (matches reference)

TRACE = False
LAST_EXEC_NS = None
LAST_TRACE = None
LAST_PROFILE = None
_CACHED = {}


def build_kernel():
    nc = bacc.Bacc("TRN2", target_bir_lowering=False, debug=False, num_devices=8)

    xT_d = nc.dram_tensor("xt", [128, B * NCHUNK * KT * 512], BF16,
                          kind="ExternalInput").ap()
    # wq/wk/wv are host-pre-arranged to the SBUF layout [128, KT*128]
    # (wX[p, kt*128+m] = W[kt*128+p, m]) so their DMA is fully contiguous
    wq_d = nc.dram_tensor("wq", [128, KT * 128], BF16, kind="ExternalInput").ap()
    wk_d = nc.dram_tensor("wk", [128, KT * 128], BF16, kind="ExternalInput").ap()
    wv_d = nc.dram_tensor("wv", [128, KT * 128], BF16, kind="ExternalInput").ap()
    wo_d = nc.dram_tensor("wo", [128, DIM], BF16, kind="ExternalInput").ap()
    out_d = nc.dram_tensor("out", [B * T, DIM], FP16, kind="ExternalOutput").ap()

    xsrc = xT_d.rearrange("p (u kt t) -> p u kt t", kt=KT, t=512)

    with tile.TileContext(nc) as tc, ExitStack() as ctx:
        cp = ctx.enter_context(tc.tile_pool(name="const", bufs=1))
        xT_p = ctx.enter_context(tc.tile_pool(name="xT", bufs=2))
        qT_p = ctx.enter_context(tc.tile_pool(name="qT", bufs=2))
        kT_p = ctx.enter_context(tc.tile_pool(name="kT", bufs=2))
        vaug_p = ctx.enter_context(tc.tile_pool(name="vaug", bufs=2))
        attnT_p = ctx.enter_context(tc.tile_pool(name="attnT", bufs=4))
        recip_p = ctx.enter_context(tc.tile_pool(name="recip", bufs=4))
        avsb_p = ctx.enter_context(tc.tile_pool(name="avsb", bufs=4))
        rbc_p = ctx.enter_context(tc.tile_pool(name="rbc", bufs=2))
        outT_p = ctx.enter_context(tc.tile_pool(name="outT", bufs=2))
        osb_p = ctx.enter_context(tc.tile_pool(name="osb", bufs=3))
        mm_ps = ctx.enter_context(tc.tile_pool(name="mmps", bufs=2, space="PSUM"))
        dots_ps = ctx.enter_context(tc.tile_pool(name="dotsps", bufs=1, space="PSUM"))
        av_ps_p = ctx.enter_context(tc.tile_pool(name="avps", bufs=2, space="PSUM"))

        # ---- weights (wq/wk first: the first q/k projections only wait on
        # them + the first x slice; wv/wo are DMA'd after the first x slices
        # via post_dma so the pipeline starts ~4us earlier) ----
        wq_sb = cp.tile([128, KT * 128], BF16, tag="wq")
        wk_sb = cp.tile([128, KT * 128], BF16, tag="wk")
        wv_sb = cp.tile([128, KT * 128], BF16, tag="wv")
        wo_sb = cp.tile([128, DIM], BF16, tag="wo")
        for w_sb, w_d in ((wq_sb, wq_d), (wk_sb, wk_d)):
            nc.sync.dma_start(w_sb[:], w_d)

        def dma_wv_wo():
            nc.sync.dma_start(wv_sb[:], wv_d)
            nc.sync.dma_start(wo_sb[:], wo_d)

        # ---- constants ----
        ones32 = cp.tile([128, NT], F32, tag="ones32")
        nc.gpsimd.memset(ones32[:], 1.0)
        onesv = ones32[:].rearrange("p (u o) -> p u o", o=1)

        state = {}     # b -> (qT, kT, vaug, outT)
        xstate = {}    # b -> xTv view (allocated 2 slices ahead of use)
        deferred = []  # (outT, t0, tt) output-projection backlog

        def ensure_xT(b2):
            if b2 not in xstate:
                xT = xT_p.tile([128, NCHUNK * KT * 512], BF16, tag="xT",
                               name="xT")
                xstate[b2] = xT[:].rearrange("p (c kt t) -> p c kt t",
                                             c=NCHUNK, kt=KT)

        def dma_slice_u(u):
            b2, c2 = divmod(u, NCHUNK)
            if b2 >= B:
                return
            ensure_xT(b2)
            nc.sync.dma_start(xstate[b2][:, c2, :, :], xsrc[:, u, :, :])

        def emit_oproj(outT, t0, tt):
            osb = osb_p.tile([128, DIM], FP16, tag="osb", name="osb")
            for half in (0, 1):
                po = mm_ps.tile([128, 512], F32, tag="mm", name="po")
                nc.tensor.matmul(po[:], outT[:, tt * 128:(tt + 1) * 128],
                                 wo_sb[:, half * 512:(half + 1) * 512],
                                 start=True, stop=True)
                nc.vector.tensor_copy(osb[:, half * 512:(half + 1) * 512],
                                      po[:])
            nc.sync.dma_start(out_d[t0 + tt * 128: t0 + (tt + 1) * 128, :],
                              osb[:])

        def p12_chunk_steps(b, ch, post_dma=None):
            """xT DMA + q/k (feat-major) + v (token-major) for chunk ch of
            batch b."""
            u = b * NCHUNK + ch
            if ch == 0:
                ensure_xT(b)
                qT = qT_p.tile([128, T], BF16, tag="qT", name="qT")
                kTt = kT_p.tile([128, T], BF16, tag="kT", name="kT")
                vaug = vaug_p.tile([128, NT * 130], BF16, tag="vaug", name="vaug")
                outT = outT_p.tile([128, T], BF16, tag="outT", name="outT")
                state[b] = (qT, kTt, vaug, outT)
            qT, kTt, vaug, outT = state[b]
            xTv = xstate[b]

            # keep the x DMA two chunk-slots ahead of the projections
            # (cross-batch too, so a batch's first chunk is ready on arrival)
            if u == 0:
                dma_slice_u(0)
                dma_slice_u(1)
                if post_dma is not None:
                    post_dma()
            dma_slice_u(u + 2)
            yield
            # q and k projections for this 512-token chunk; yield
            # mid-accumulation so the PE work spreads across the
            # interleaved attention pair steps
            for w_sb, dest in ((wq_sb, qT), (wk_sb, kTt)):
                pp = mm_ps.tile([128, 512], F32, tag="mm", name="pp")
                for kt in range(KT):
                    nc.tensor.matmul(pp[:], w_sb[:, kt * 128:(kt + 1) * 128],
                                     xTv[:, ch, kt, :],
                                     start=(kt == 0), stop=(kt == KT - 1))
                    if kt == 3:
                        yield
                nc.vector.tensor_copy(dest[:, ch * 512:(ch + 1) * 512], pp[:])
                yield
            # v for the same 4 token-tiles, token-major
            vv = vaug[:].rearrange("p (jt c) -> p jt c", c=130)
            vp = mm_ps.tile([128, 512], F32, tag="mm", name="vp")
            for j in range(4):
                for kt in range(KT):
                    nc.tensor.matmul(
                        vp[:, j * 128:(j + 1) * 128],
                        xTv[:, ch, kt, j * 128:(j + 1) * 128],
                        wv_sb[:, kt * 128:(kt + 1) * 128],
                        start=(kt == 0), stop=(kt == KT - 1))
                if j == 1:
                    yield
            src = vp[:].rearrange("p (j c) -> p j c", j=4)
            nc.vector.tensor_copy(vv[:, 4 * ch:4 * ch + 4, 0:64], src[:, :, 0:64])
            nc.vector.tensor_copy(vv[:, 4 * ch:4 * ch + 4, 65:129], src[:, :, 64:128])
            # ones columns via DVE (gpsimd's in-order queue is slow on
            # semaphore ops and would delay the normalize broadcast)
            nc.vector.tensor_copy(vv[:, 4 * ch:4 * ch + 4, 64:65],
                                  onesv[:, 4 * ch:4 * ch + 4, :])
            nc.vector.tensor_copy(vv[:, 4 * ch:4 * ch + 4, 129:130],
                                  onesv[:, 4 * ch:4 * ch + 4, :])
            yield

        def att_chunk_steps(b, c):
            """Attention chunk c of batch b + deferred output projections."""
            t0 = b * T
            qT, kTt, vaug, outT = state[b]
            njt = 4 * (c + 1)
            avp = {h: av_ps_p.tile([65, 512], F32, tag="av", name=f"avp{h}")
                   for h in (0, 1)}

            def emit_av(pend):
                jts, offs, at = pend
                for h in (0, 1):
                    for j, jt in enumerate(jts):
                        off = offs[j]
                        nc.tensor.matmul(
                            avp[h][:, off:512],
                            vaug[:, jt * 130 + 65 * h: jt * 130 + 65 * h + 65],
                            at[:, h * 1024 + j * 512 + off: h * 1024 + (j + 1) * 512],
                            start=(jt == 0), stop=(jt == njt - 1))

            pend = None  # exp'd pair awaiting its AV (one-pair software pipeline)
            for jp in range(njt // 2):
                jts = (2 * jp, 2 * jp + 1)
                offs = [max(512 * c, jt * 128) - 512 * c for jt in jts]
                # one psum tile per pair: h0 cols [0:1024], h1 [1024:2048]
                dps = dots_ps.tile([128, 2048], F32, tag="dots", name="dp")
                # j outer, h inner: adjacent matmuls hit disjoint PE row
                # groups (h0 rows 0-63, h1 rows 64-127) and run concurrently
                for j, jt in enumerate(jts):
                    off = offs[j]
                    for h in (0, 1):
                        nc.tensor.matmul(
                            dps[:, h * 1024 + j * 512 + off: h * 1024 + (j + 1) * 512],
                            kTt[64 * h:64 * h + 64, jt * 128:(jt + 1) * 128],
                            qT[64 * h:64 * h + 64, 512 * c + off:512 * (c + 1)],
                            start=True, stop=True)
                # single exp over both heads' scores, emitted BEFORE the
                # previous pair's AV: it only waits on the 0.2us scores
                # matmuls, so ACT exps chain back-to-back while the PE runs
                # AV(p-1) + oproj + projection filler underneath
                at = attnT_p.tile([128, 2048], BF16, tag="at", name="at")
                nc.scalar.activation(at[:, offs[0]:2048], dps[:, offs[0]:2048],
                                     mybir.ActivationFunctionType.Exp,
                                     bias=0.0, scale=float(SCALE))
                for h in (0, 1):
                    for j, jt in enumerate(jts):
                        if jt >= 4 * c:  # zero invalid (j > i) entries
                            base = h * 1024 + j * 512 + offs[j]
                            nc.gpsimd.affine_select(
                                out=at[:, base: base + 128],
                                in_=at[:, base: base + 128],
                                compare_op=mybir.AluOpType.is_ge, fill=0.0,
                                base=0, pattern=[[1, 128]], channel_multiplier=-1)
                if pend is not None:
                    emit_av(pend)
                # oproj backlog as PE filler, but hold back 3 entries for
                # the chunk tail (the last pair's exp + the normalize chain
                # have no scores/AV to hide under)
                if len(deferred) > 3:
                    emit_oproj(*deferred.pop(0))
                pend = (jts, offs, at)
                yield
            # chunk tail: reserved oprojs go to the PE queue BEFORE the last
            # pair's AV (which waits on its exp) - the in-order PE drains
            # them while ACT finishes
            for _ in range(3):
                if deferred:
                    emit_oproj(*deferred.pop(0))
            emit_av(pend)
            if deferred:
                emit_oproj(*deferred.pop(0))
            yield
            # normalize (fast recip keeps this short); the dependent oproj
            # is deferred into the next chunk's pairs.  NOTE: den must be
            # staged to a partition-0 tile - both the custom-DVE reciprocal
            # and any op pair with mismatched partition offsets misbehave.
            for h in (0, 1):
                den = recip_p.tile([1, 512], F32, tag="den", name="den")
                nc.vector.tensor_copy(den[:], avp[h][64:65, :])
                rc = recip_p.tile([1, 512], F32, tag="recip", name="rc")
                nc.vector.reciprocal_approx_fast(rc[:], den[:])
                rb = rbc_p.tile([64, 512], F32, tag="rbc", name="rb")
                nc.gpsimd.partition_broadcast(rb[:], rc[:])
                nc.vector.scalar_tensor_tensor(
                    outT[64 * h:64 * h + 64, c * 512:(c + 1) * 512],
                    avp[h][0:64, :], 1.0, rb[:],
                    op0=mybir.AluOpType.mult, op1=mybir.AluOpType.mult)
            deferred.extend((outT, t0, tt) for tt in range(4 * c, 4 * c + 4))
            yield

        def drive(gens):
            """Credit-weighted round-robin of (generator, weight) pairs: a
            generator advances ~weight steps per round, so both exhaust at
            the same time and the emission (= in-order engine queue order)
            keeps independent work spread between the dependency chains."""
            gens = [gw for gw in gens if gw is not None and gw[0] is not None]
            credit = [0.0] * len(gens)
            alive = [True] * len(gens)
            while any(alive):
                for i, (g, w) in enumerate(gens):
                    if not alive[i]:
                        continue
                    credit[i] += w
                    while credit[i] >= 1.0 and alive[i]:
                        credit[i] -= 1.0
                        try:
                            next(g)
                        except StopIteration:
                            alive[i] = False

        P12_STEPS = 7.0
        units = [(b, ch) for b in range(B) for ch in range(NCHUNK)]
        prev = None
        for i, (b, ch) in enumerate(units):
            att = att_chunk_steps(*prev) if prev is not None else None
            att_steps = 2 * (prev[1] + 1) + 2 if prev is not None else 1
            drive([(att, 1.0) if att is not None else None,
                   (p12_chunk_steps(b, ch, post_dma=dma_wv_wo if i == 0 else None),
                    P12_STEPS / att_steps if att is not None else 4.0)])
            prev = (b, ch)
        drive([(att_chunk_steps(*prev), 1.0)])
        while deferred:
            emit_oproj(*deferred.pop(0))

    nc.compile()
    return nc


def kernel(x, Wq, Wkv, Wout, bout):
    """Full inputs -> full output. Shards across 8 NeuronCores internally."""
    global LAST_EXEC_NS, LAST_TRACE
    if "nc" not in _CACHED:
        _CACHED["nc"] = build_kernel()
    nc = _CACHED["nc"]

    hdt = ml_dtypes.bfloat16
    xf = np.asarray(x, dtype=np.float32).reshape(B, NCHUNK, 512, DIM)
    # [128, B, NCHUNK, KT, 512]: per-(chunk,partition) rows are 8KB
    # contiguous in DRAM so the per-chunk DMA moves full-size packets
    xT = np.ascontiguousarray(
        xf.transpose(3, 0, 1, 2)            # [DIM, B, NCHUNK, 512]
          .reshape(KT, 128, B, NCHUNK, 512)
          .transpose(1, 2, 3, 0, 4)).astype(hdt)
    xT = xT.reshape(128, B * NCHUNK * KT * 512)
    Wq = np.asarray(Wq, dtype=np.float32).astype(hdt)
    Wkv = np.asarray(Wkv, dtype=np.float32).astype(hdt)
    Wout = np.asarray(Wout, dtype=np.float32).astype(hdt)
    bout = np.asarray(bout, dtype=np.float32)

    def wlayout(w):  # [DIM, 128] -> [128, KT*128] SBUF layout for clean DMA
        return np.ascontiguousarray(
            w.reshape(KT, 128, 128).transpose(1, 0, 2).reshape(128, KT * 128))

    in_maps = []
    for c in range(8):
        s = slice(128 * c, 128 * (c + 1))
        in_maps.append({
            "xt": xT,
            "wq": wlayout(Wq[:, s]),
            "wk": wlayout(Wkv[:, :DIM][:, s]),
            "wv": wlayout(Wkv[:, DIM:][:, s]),
            "wo": np.ascontiguousarray(Wout[s, :]),
        })

    res = bass_utils.run_bass_kernel_spmd(nc, in_maps, core_ids=list(range(8)),
                                          trace=TRACE)
    if TRACE:
        LAST_EXEC_NS = res.exec_time_ns
        LAST_TRACE = res.instructions_and_trace
        globals()["LAST_PROFILE"] = getattr(res, "profile_json", None)
    acc = res.results[0]["out"].astype(np.float64)
    for c in range(1, 8):
        acc += res.results[c]["out"]
    out = (acc + bout.astype(np.float64)).astype(np.float32)
    return out.reshape(B, T, DIM)


# revision 18
# speedup vs baseline: 1.1632x; 1.0409x over previous
"""Causal multi-head attention block (16 heads, dim 1024) on 8 TRN2 NeuronCores.

Sharding: tensor-parallel over heads - core c computes heads {2c, 2c+1}:
  q/k/v projections with the 128-column weight slices, causal attention,
  and a partial output projection with the matching 128 Wout rows.
Host sums the 8 partial outputs and adds the bias.

Design notes (v2 - per-chunk software pipeline):
  * The host supplies x PRE-TRANSPOSED and chunk-blocked
    (xT [128, B*NCHUNK, KT, 512] bf16) so each per-chunk DMA reads 8KB
    contiguous per partition (the flat [dim, b*n] layout produced ~600B
    packets and 130us of DMA active time).
  * q/k are computed feature-major (qT/kT [feat, tok]) for the score
    matmuls: lhsT = W slice, rhs = xT chunk (N=512 streams). v is computed
    TOKEN-major directly -> vaug ([128 j, 65] per j-tile per head: 64 v
    cols + a ones col that makes the AV matmul also produce softmax
    denominators).
  * scores TRANSPOSED: dotsT[j,i] = kT.T @ qT per (j-tile, head), K=64 ->
    the two heads go to PE row groups (0,0)/(64,0) and run CONCURRENTLY;
    both land in one [128, 2048] psum tile so a single ACT exp op covers a
    j-tile pair x both heads. Diagonal tiles are zeroed above the diagonal
    with gpsimd affine_select.
  * Pipeline granularity is a (batch, 512-token chunk) unit: attention for
    unit u runs interleaved (via generator round-robin) with projections
    for unit u+1, so the PE always has independent matmul work queued
    while ACT runs the 1.9us exp of a score pair. This kills the 26us
    startup bubble and the ACT-bound tail the batch-level pipeline had
    (the PE also drops out of its 2.4GHz p-state after ~any stall, so
    gaps cost ~2x their length).
  * Within a pair iteration the emission order is scores(p) -> exp(p) ->
    AV(p-1) -> deferred oproj: exp(p) only waits on the 0.2us scores
    matmul, so consecutive exps chain back-to-back on ACT while the PE
    covers AV + oproj + projection filler under them.
  * The output projection of chunk c is deferred into chunk c+1's pair
    loop so the normalize chain (SBUF-staged reciprocal_approx_fast +
    partition_broadcast + stt) is never on the PE's critical path.
  * All DMA stays on nc.sync: issuing DMA from scalar/gpsimd blocks those
    engines' in-order queues on the DMA's dependencies (measured -10%).
"""
import numpy as np
import ml_dtypes
from contextlib import ExitStack

import concourse.bacc as bacc
import concourse.mybir as mybir
import concourse.tile as tile
import concourse.bass_utils as bass_utils

F32 = mybir.dt.float32
BF16 = mybir.dt.bfloat16
FP16 = mybir.dt.float16

B = 4            # batches
T = 2048         # tokens per batch
DIM = 1024
NT = T // 128    # token tiles per batch (16)
KT = DIM // 128  # contraction tiles (8)
NCHUNK = T // 512  # 512-col i-chunks per batch (4)
SCALE = DIM ** -0.5  # 1/32 - NOTE: full dim, not head dim (matches reference)

TRACE = False
LAST_EXEC_NS = None
LAST_TRACE = None
LAST_PROFILE = None
_CACHED = {}


def build_kernel():
    nc = bacc.Bacc("TRN2", target_bir_lowering=False, debug=False, num_devices=8)

    xT_d = nc.dram_tensor("xt", [128, B * NCHUNK * KT * 512], BF16,
                          kind="ExternalInput").ap()
    # wq/wk/wv are host-pre-arranged to the SBUF layout [128, KT*128]
    # (wX[p, kt*128+m] = W[kt*128+p, m]) so their DMA is fully contiguous
    wq_d = nc.dram_tensor("wq", [128, KT * 128], BF16, kind="ExternalInput").ap()
    wk_d = nc.dram_tensor("wk", [128, KT * 128], BF16, kind="ExternalInput").ap()
    wv_d = nc.dram_tensor("wv", [128, KT * 128], BF16, kind="ExternalInput").ap()
    wo_d = nc.dram_tensor("wo", [128, DIM], BF16, kind="ExternalInput").ap()
    out_d = nc.dram_tensor("out", [B * T, DIM], FP16, kind="ExternalOutput").ap()

    xsrc = xT_d.rearrange("p (u kt t) -> p u kt t", kt=KT, t=512)

    with tile.TileContext(nc) as tc, ExitStack() as ctx:
        cp = ctx.enter_context(tc.tile_pool(name="const", bufs=1))
        xT_p = ctx.enter_context(tc.tile_pool(name="xT", bufs=2))
        qT_p = ctx.enter_context(tc.tile_pool(name="qT", bufs=2))
        kT_p = ctx.enter_context(tc.tile_pool(name="kT", bufs=2))
        vaug_p = ctx.enter_context(tc.tile_pool(name="vaug", bufs=2))
        attnT_p = ctx.enter_context(tc.tile_pool(name="attnT", bufs=6))
        recip_p = ctx.enter_context(tc.tile_pool(name="recip", bufs=4))
        avsb_p = ctx.enter_context(tc.tile_pool(name="avsb", bufs=4))
        rbc_p = ctx.enter_context(tc.tile_pool(name="rbc", bufs=2))
        outT_p = ctx.enter_context(tc.tile_pool(name="outT", bufs=2))
        osb_p = ctx.enter_context(tc.tile_pool(name="osb", bufs=3))
        mm_ps = ctx.enter_context(tc.tile_pool(name="mmps", bufs=2, space="PSUM"))
        dots_ps = ctx.enter_context(tc.tile_pool(name="dotsps", bufs=2, space="PSUM"))
        av_ps_p = ctx.enter_context(tc.tile_pool(name="avps", bufs=2, space="PSUM"))

        # ---- weights (wq/wk first: the first q/k projections only wait on
        # them + the first x slice; wv/wo are DMA'd after the first x slices
        # via post_dma so the pipeline starts ~4us earlier) ----
        wq_sb = cp.tile([128, KT * 128], BF16, tag="wq")
        wk_sb = cp.tile([128, KT * 128], BF16, tag="wk")
        wv_sb = cp.tile([128, KT * 128], BF16, tag="wv")
        wo_sb = cp.tile([128, DIM], BF16, tag="wo")
        for w_sb, w_d in ((wq_sb, wq_d), (wk_sb, wk_d)):
            nc.sync.dma_start(w_sb[:], w_d)

        def dma_wv_wo():
            nc.sync.dma_start(wv_sb[:], wv_d)
            nc.sync.dma_start(wo_sb[:], wo_d)

        # ---- constants ----
        ones32 = cp.tile([128, NT], F32, tag="ones32")
        nc.gpsimd.memset(ones32[:], 1.0)
        onesv = ones32[:].rearrange("p (u o) -> p u o", o=1)

        state = {}     # b -> (qT, kT, vaug, outT)
        xstate = {}    # b -> xTv view (allocated 2 slices ahead of use)
        deferred = []  # (outT, t0, tt) output-projection backlog

        def ensure_xT(b2):
            if b2 not in xstate:
                xT = xT_p.tile([128, NCHUNK * KT * 512], BF16, tag="xT",
                               name="xT")
                xstate[b2] = xT[:].rearrange("p (c kt t) -> p c kt t",
                                             c=NCHUNK, kt=KT)

        def dma_slice_u(u):
            b2, c2 = divmod(u, NCHUNK)
            if b2 >= B:
                return
            ensure_xT(b2)
            nc.sync.dma_start(xstate[b2][:, c2, :, :], xsrc[:, u, :, :])

        def emit_oproj(outT, t0, tt):
            osb = osb_p.tile([128, DIM], FP16, tag="osb", name="osb")
            for half in (0, 1):
                po = mm_ps.tile([128, 512], F32, tag="mm", name="po")
                nc.tensor.matmul(po[:], outT[:, tt * 128:(tt + 1) * 128],
                                 wo_sb[:, half * 512:(half + 1) * 512],
                                 start=True, stop=True)
                nc.vector.tensor_copy(osb[:, half * 512:(half + 1) * 512],
                                      po[:])
            nc.sync.dma_start(out_d[t0 + tt * 128: t0 + (tt + 1) * 128, :],
                              osb[:])

        def p12_chunk_steps(b, ch, post_dma=None):
            """xT DMA + q/k (feat-major) + v (token-major) for chunk ch of
            batch b."""
            u = b * NCHUNK + ch
            if ch == 0:
                ensure_xT(b)
                qT = qT_p.tile([128, T], BF16, tag="qT", name="qT")
                kTt = kT_p.tile([128, T], BF16, tag="kT", name="kT")
                vaug = vaug_p.tile([128, NT * 130], BF16, tag="vaug", name="vaug")
                outT = outT_p.tile([128, T], BF16, tag="outT", name="outT")
                state[b] = (qT, kTt, vaug, outT)
            qT, kTt, vaug, outT = state[b]
            xTv = xstate[b]

            # keep the x DMA two chunk-slots ahead of the projections
            # (cross-batch too, so a batch's first chunk is ready on arrival)
            if u == 0:
                dma_slice_u(0)
                dma_slice_u(1)
                if post_dma is not None:
                    post_dma()
            dma_slice_u(u + 2)
            yield
            # q and k projections for this 512-token chunk; yield
            # mid-accumulation so the PE work spreads across the
            # interleaved attention pair steps
            for w_sb, dest in ((wq_sb, qT), (wk_sb, kTt)):
                pp = mm_ps.tile([128, 512], F32, tag="mm", name="pp")
                for kt in range(KT):
                    nc.tensor.matmul(pp[:], w_sb[:, kt * 128:(kt + 1) * 128],
                                     xTv[:, ch, kt, :],
                                     start=(kt == 0), stop=(kt == KT - 1))
                    if kt == 3:
                        yield
                nc.vector.tensor_copy(dest[:, ch * 512:(ch + 1) * 512], pp[:])
                yield
            # v for the same 4 token-tiles, token-major
            vv = vaug[:].rearrange("p (jt c) -> p jt c", c=130)
            vp = mm_ps.tile([128, 512], F32, tag="mm", name="vp")
            for j in range(4):
                for kt in range(KT):
                    nc.tensor.matmul(
                        vp[:, j * 128:(j + 1) * 128],
                        xTv[:, ch, kt, j * 128:(j + 1) * 128],
                        wv_sb[:, kt * 128:(kt + 1) * 128],
                        start=(kt == 0), stop=(kt == KT - 1))
                if j == 1:
                    yield
            src = vp[:].rearrange("p (j c) -> p j c", j=4)
            nc.vector.tensor_copy(vv[:, 4 * ch:4 * ch + 4, 0:64], src[:, :, 0:64])
            nc.vector.tensor_copy(vv[:, 4 * ch:4 * ch + 4, 65:129], src[:, :, 64:128])
            # ones columns via DVE (gpsimd's in-order queue is slow on
            # semaphore ops and would delay the normalize broadcast)
            nc.vector.tensor_copy(vv[:, 4 * ch:4 * ch + 4, 64:65],
                                  onesv[:, 4 * ch:4 * ch + 4, :])
            nc.vector.tensor_copy(vv[:, 4 * ch:4 * ch + 4, 129:130],
                                  onesv[:, 4 * ch:4 * ch + 4, :])
            yield

        def att_chunk_steps(b, c):
            """Attention chunk c of batch b + deferred output projections."""
            t0 = b * T
            qT, kTt, vaug, outT = state[b]
            njt = 4 * (c + 1)
            avp = {h: av_ps_p.tile([65, 512], F32, tag="av", name=f"avp{h}")
                   for h in (0, 1)}

            def emit_av(pend):
                jt, off, at = pend
                for h in (0, 1):
                    nc.tensor.matmul(
                        avp[h][:, off:512],
                        vaug[:, jt * 130 + 65 * h: jt * 130 + 65 * h + 65],
                        at[:, h * 512 + off: (h + 1) * 512],
                        start=(jt == 0), stop=(jt == njt - 1))

            # one [128,1024] dots tile PER J-TILE from a double-buffered
            # pool: scores(jt+1) write the other buffer while exp(jt) reads,
            # so the PE NEVER waits on ACT for scores (no psum WAR).  The AV
            # matmuls lag 2 j-tiles so their exp is long done when the
            # in-order PE reaches them.
            pends = []
            for jt in range(njt):
                off = max(512 * c, jt * 128) - 512 * c
                dps = dots_ps.tile([128, 1024], F32, tag="dots", name="dp")
                # h0/h1 hit disjoint PE row groups (rows 0-63 / 64-127) and
                # run concurrently
                for h in (0, 1):
                    nc.tensor.matmul(
                        dps[:, h * 512 + off: (h + 1) * 512],
                        kTt[64 * h:64 * h + 64, jt * 128:(jt + 1) * 128],
                        qT[64 * h:64 * h + 64, 512 * c + off:512 * (c + 1)],
                        start=True, stop=True)
                at = attnT_p.tile([128, 1024], BF16, tag="at", name="at")
                # exp only the valid columns of both heads (strided AP)
                dv = dps[:].rearrange("p (h i) -> p h i", h=2)
                atv = at[:].rearrange("p (h i) -> p h i", h=2)
                nc.scalar.activation(atv[:, :, off:512], dv[:, :, off:512],
                                     mybir.ActivationFunctionType.Exp,
                                     bias=0.0, scale=float(SCALE))
                if jt >= 4 * c:  # zero invalid (j > i) entries of the diag tile
                    for h in (0, 1):
                        base = h * 512 + off
                        nc.gpsimd.affine_select(
                            out=at[:, base: base + 128],
                            in_=at[:, base: base + 128],
                            compare_op=mybir.AluOpType.is_ge, fill=0.0,
                            base=0, pattern=[[1, 128]], channel_multiplier=-1)
                if len(pends) >= 2:
                    emit_av(pends.pop(0))
                # oproj backlog as PE filler; hold back 2 for the chunk tail
                if len(deferred) > 2:
                    emit_oproj(*deferred.pop(0))
                pends.append((jt, off, at))
                yield
            # chunk tail: reserved oprojs interleave with the trailing AVs
            while pends:
                if deferred:
                    emit_oproj(*deferred.pop(0))
                emit_av(pends.pop(0))
            yield
            # normalize (fast recip keeps this short); the dependent oproj
            # is deferred into the next chunk's pairs.  NOTE: den must be
            # staged to a partition-0 tile - both the custom-DVE reciprocal
            # and any op pair with mismatched partition offsets misbehave.
            for h in (0, 1):
                den = recip_p.tile([1, 512], F32, tag="den", name="den")
                nc.vector.tensor_copy(den[:], avp[h][64:65, :])
                rc = recip_p.tile([1, 512], F32, tag="recip", name="rc")
                nc.vector.reciprocal_approx_fast(rc[:], den[:])
                rb = rbc_p.tile([64, 512], F32, tag="rbc", name="rb")
                nc.gpsimd.partition_broadcast(rb[:], rc[:])
                nc.vector.scalar_tensor_tensor(
                    outT[64 * h:64 * h + 64, c * 512:(c + 1) * 512],
                    avp[h][0:64, :], 1.0, rb[:],
                    op0=mybir.AluOpType.mult, op1=mybir.AluOpType.mult)
            deferred.extend((outT, t0, tt) for tt in range(4 * c, 4 * c + 4))
            yield

        def drive(gens):
            """Credit-weighted round-robin of (generator, weight) pairs: a
            generator advances ~weight steps per round, so both exhaust at
            the same time and the emission (= in-order engine queue order)
            keeps independent work spread between the dependency chains."""
            gens = [gw for gw in gens if gw is not None and gw[0] is not None]
            credit = [0.0] * len(gens)
            alive = [True] * len(gens)
            while any(alive):
                for i, (g, w) in enumerate(gens):
                    if not alive[i]:
                        continue
                    credit[i] += w
                    while credit[i] >= 1.0 and alive[i]:
                        credit[i] -= 1.0
                        try:
                            next(g)
                        except StopIteration:
                            alive[i] = False

        P12_STEPS = 7.0
        units = [(b, ch) for b in range(B) for ch in range(NCHUNK)]
        prev = None
        for i, (b, ch) in enumerate(units):
            att = att_chunk_steps(*prev) if prev is not None else None
            att_steps = 4 * (prev[1] + 1) + 2 if prev is not None else 1
            drive([(att, 1.0) if att is not None else None,
                   (p12_chunk_steps(b, ch, post_dma=dma_wv_wo if i == 0 else None),
                    P12_STEPS / att_steps if att is not None else 4.0)])
            prev = (b, ch)
        drive([(att_chunk_steps(*prev), 1.0)])
        while deferred:
            emit_oproj(*deferred.pop(0))

    nc.compile()
    return nc


def kernel(x, Wq, Wkv, Wout, bout):
    """Full inputs -> full output. Shards across 8 NeuronCores internally."""
    global LAST_EXEC_NS, LAST_TRACE
    if "nc" not in _CACHED:
        _CACHED["nc"] = build_kernel()
    nc = _CACHED["nc"]

    hdt = ml_dtypes.bfloat16
    xf = np.asarray(x, dtype=np.float32).reshape(B, NCHUNK, 512, DIM)
    # [128, B, NCHUNK, KT, 512]: per-(chunk,partition) rows are 8KB
    # contiguous in DRAM so the per-chunk DMA moves full-size packets
    xT = np.ascontiguousarray(
        xf.transpose(3, 0, 1, 2)            # [DIM, B, NCHUNK, 512]
          .reshape(KT, 128, B, NCHUNK, 512)
          .transpose(1, 2, 3, 0, 4)).astype(hdt)
    xT = xT.reshape(128, B * NCHUNK * KT * 512)
    Wq = np.asarray(Wq, dtype=np.float32).astype(hdt)
    Wkv = np.asarray(Wkv, dtype=np.float32).astype(hdt)
    Wout = np.asarray(Wout, dtype=np.float32).astype(hdt)
    bout = np.asarray(bout, dtype=np.float32)

    def wlayout(w):  # [DIM, 128] -> [128, KT*128] SBUF layout for clean DMA
        return np.ascontiguousarray(
            w.reshape(KT, 128, 128).transpose(1, 0, 2).reshape(128, KT * 128))

    in_maps = []
    for c in range(8):
        s = slice(128 * c, 128 * (c + 1))
        in_maps.append({
            "xt": xT,
            "wq": wlayout(Wq[:, s]),
            "wk": wlayout(Wkv[:, :DIM][:, s]),
            "wv": wlayout(Wkv[:, DIM:][:, s]),
            "wo": np.ascontiguousarray(Wout[s, :]),
        })

    res = bass_utils.run_bass_kernel_spmd(nc, in_maps, core_ids=list(range(8)),
                                          trace=TRACE)
    if TRACE:
        LAST_EXEC_NS = res.exec_time_ns
        LAST_TRACE = res.instructions_and_trace
        globals()["LAST_PROFILE"] = getattr(res, "profile_json", None)
    acc = res.results[0]["out"].astype(np.float64)
    for c in range(1, 8):
        acc += res.results[c]["out"]
    out = (acc + bout.astype(np.float64)).astype(np.float32)
    return out.reshape(B, T, DIM)


# revision 25
# speedup vs baseline: 1.2032x; 1.0344x over previous
"""Causal multi-head attention block (16 heads, dim 1024) on 8 TRN2 NeuronCores.

Sharding: tensor-parallel over heads - core c computes heads {2c, 2c+1}:
  q/k/v projections with the 128-column weight slices, causal attention,
  and a partial output projection with the matching 128 Wout rows.
Host sums the 8 partial outputs and adds the bias.

Design notes (v2 - per-chunk software pipeline):
  * The host supplies x PRE-TRANSPOSED and chunk-blocked
    (xT [128, B*NCHUNK, KT, 512] bf16) so each per-chunk DMA reads 8KB
    contiguous per partition (the flat [dim, b*n] layout produced ~600B
    packets and 130us of DMA active time).
  * q/k are computed feature-major (qT/kT [feat, tok]) for the score
    matmuls: lhsT = W slice, rhs = xT chunk (N=512 streams). v is computed
    TOKEN-major directly -> vaug ([128 j, 65] per j-tile per head: 64 v
    cols + a ones col that makes the AV matmul also produce softmax
    denominators).
  * scores TRANSPOSED: dotsT[j,i] = kT.T @ qT per (j-tile, head), K=64 ->
    the two heads go to PE row groups (0,0)/(64,0) and run CONCURRENTLY;
    both land in one [128, 2048] psum tile so a single ACT exp op covers a
    j-tile pair x both heads. Diagonal tiles are zeroed above the diagonal
    with gpsimd affine_select.
  * Pipeline granularity is a (batch, 512-token chunk) unit: attention for
    unit u runs interleaved (via generator round-robin) with projections
    for unit u+1, so the PE always has independent matmul work queued
    while ACT runs the 1.9us exp of a score pair. This kills the 26us
    startup bubble and the ACT-bound tail the batch-level pipeline had
    (the PE also drops out of its 2.4GHz p-state after ~any stall, so
    gaps cost ~2x their length).
  * Within a pair iteration the emission order is scores(p) -> exp(p) ->
    AV(p-1) -> deferred oproj: exp(p) only waits on the 0.2us scores
    matmul, so consecutive exps chain back-to-back on ACT while the PE
    covers AV + oproj + projection filler under them.
  * The output projection of chunk c is deferred into chunk c+1's pair
    loop so the normalize chain (SBUF-staged reciprocal_approx_fast +
    partition_broadcast + stt) is never on the PE's critical path.
  * All DMA stays on nc.sync: issuing DMA from scalar/gpsimd blocks those
    engines' in-order queues on the DMA's dependencies (measured -10%).
"""
import numpy as np
import ml_dtypes
from contextlib import ExitStack

import concourse.bacc as bacc
import concourse.mybir as mybir
import concourse.tile as tile
import concourse.bass_utils as bass_utils

F32 = mybir.dt.float32
BF16 = mybir.dt.bfloat16
FP16 = mybir.dt.float16

B = 4            # batches
T = 2048         # tokens per batch
DIM = 1024
NT = T // 128    # token tiles per batch (16)
KT = DIM // 128  # contraction tiles (8)
NCHUNK = T // 512  # 512-col i-chunks per batch (4)
SCALE = DIM ** -0.5  # 1/32 - NOTE: full dim, not head dim (matches reference)

TRACE = False
LAST_EXEC_NS = None
LAST_TRACE = None
LAST_PROFILE = None
_CACHED = {}


def build_kernel():
    nc = bacc.Bacc("TRN2", target_bir_lowering=False, debug=False, num_devices=8)

    xT_d = nc.dram_tensor("xt", [128, B * NCHUNK * KT * 512], BF16,
                          kind="ExternalInput").ap()
    # wq/wk/wv are host-pre-arranged to the SBUF layout [128, KT*128]
    # (wX[p, kt*128+m] = W[kt*128+p, m]) so their DMA is fully contiguous
    wq_d = nc.dram_tensor("wq", [128, KT * 128], BF16, kind="ExternalInput").ap()
    wk_d = nc.dram_tensor("wk", [128, KT * 128], BF16, kind="ExternalInput").ap()
    wv_d = nc.dram_tensor("wv", [128, KT * 128], BF16, kind="ExternalInput").ap()
    wo_d = nc.dram_tensor("wo", [128, DIM], BF16, kind="ExternalInput").ap()
    out_d = nc.dram_tensor("out", [B * T, DIM], FP16, kind="ExternalOutput").ap()

    xsrc = xT_d.rearrange("p (u kt t) -> p u kt t", kt=KT, t=512)

    with tile.TileContext(nc) as tc, ExitStack() as ctx:
        cp = ctx.enter_context(tc.tile_pool(name="const", bufs=1))
        xT_p = ctx.enter_context(tc.tile_pool(name="xT", bufs=2))
        qT_p = ctx.enter_context(tc.tile_pool(name="qT", bufs=2))
        kT_p = ctx.enter_context(tc.tile_pool(name="kT", bufs=2))
        vaug_p = ctx.enter_context(tc.tile_pool(name="vaug", bufs=2))
        attnT_p = ctx.enter_context(tc.tile_pool(name="attnT", bufs=6))
        recip_p = ctx.enter_context(tc.tile_pool(name="recip", bufs=4))
        avsb_p = ctx.enter_context(tc.tile_pool(name="avsb", bufs=4))
        rbc_p = ctx.enter_context(tc.tile_pool(name="rbc", bufs=2))
        outT_p = ctx.enter_context(tc.tile_pool(name="outT", bufs=2))
        osb_p = ctx.enter_context(tc.tile_pool(name="osb", bufs=3))
        mm_ps = ctx.enter_context(tc.tile_pool(name="mmps", bufs=2, space="PSUM"))
        dots_ps = ctx.enter_context(tc.tile_pool(name="dotsps", bufs=2, space="PSUM"))
        av_ps_p = ctx.enter_context(tc.tile_pool(name="avps", bufs=2, space="PSUM"))

        # ---- weights.  Only wq rides the sync DMA queue (ahead of the x
        # slices); wk/wv/wo are issued from the idle vector/gpsimd/scalar
        # queues so all four transfer in parallel at startup (the issuing
        # engine only blocks on a DMA's dependencies, and weights have
        # none) ----
        wq_sb = cp.tile([128, KT * 128], BF16, tag="wq")
        wk_sb = cp.tile([128, KT * 128], BF16, tag="wk")
        wv_sb = cp.tile([128, KT * 128], BF16, tag="wv")
        wo_sb = cp.tile([128, DIM], BF16, tag="wo")
        nc.sync.dma_start(wq_sb[:], wq_d)

        def dma_wk_wv_wo():
            nc.scalar.dma_start(wk_sb[:], wk_d)
            nc.gpsimd.dma_start(wv_sb[:], wv_d)
            nc.gpsimd.dma_start(wo_sb[:], wo_d)

        # ---- constants ----
        ones32 = cp.tile([128, NT], F32, tag="ones32")
        nc.gpsimd.memset(ones32[:], 1.0)
        onesv = ones32[:].rearrange("p (u o) -> p u o", o=1)

        state = {}     # b -> (qT, kT, vaug, outT)
        xstate = {}    # b -> xTv view (allocated 2 slices ahead of use)
        deferred = []  # (outT, t0, tt) output-projection backlog

        def ensure_xT(b2):
            if b2 not in xstate:
                xT = xT_p.tile([128, NCHUNK * KT * 512], BF16, tag="xT",
                               name="xT")
                xstate[b2] = xT[:].rearrange("p (c kt t) -> p c kt t",
                                             c=NCHUNK, kt=KT)

        def dma_slice_u(u):
            b2, c2 = divmod(u, NCHUNK)
            if b2 >= B:
                return
            ensure_xT(b2)
            nc.sync.dma_start(xstate[b2][:, c2, :, :], xsrc[:, u, :, :])

        def emit_oproj(outT, t0, tt):
            osb = osb_p.tile([128, DIM], FP16, tag="osb", name="osb")
            for half in (0, 1):
                po = mm_ps.tile([128, 512], F32, tag="mm", name="po")
                nc.tensor.matmul(po[:], outT[:, tt * 128:(tt + 1) * 128],
                                 wo_sb[:, half * 512:(half + 1) * 512],
                                 start=True, stop=True)
                nc.vector.tensor_copy(osb[:, half * 512:(half + 1) * 512],
                                      po[:])
            nc.sync.dma_start(out_d[t0 + tt * 128: t0 + (tt + 1) * 128, :],
                              osb[:])

        def p12_chunk_steps(b, ch):
            """xT DMA + q/k (feat-major) + v (token-major) for chunk ch of
            batch b."""
            u = b * NCHUNK + ch
            if ch == 0:
                ensure_xT(b)
                qT = qT_p.tile([128, T], BF16, tag="qT", name="qT")
                kTt = kT_p.tile([128, T], BF16, tag="kT", name="kT")
                vaug = vaug_p.tile([128, NT * 130], BF16, tag="vaug", name="vaug")
                outT = outT_p.tile([128, T], BF16, tag="outT", name="outT")
                state[b] = (qT, kTt, vaug, outT)
            qT, kTt, vaug, outT = state[b]
            xTv = xstate[b]

            # keep the x DMA two chunk-slots ahead of the projections
            # (cross-batch too, so a batch's first chunk is ready on arrival)
            if u == 0:
                dma_wk_wv_wo()
                # first slice in two half-K DMAs: the q projection's kt 0-3
                # accumulation starts as soon as the first half lands
                for hf in (0, 1):
                    nc.sync.dma_start(xstate[0][:, 0, 4 * hf:4 * hf + 4, :],
                                      xsrc[:, 0, 4 * hf:4 * hf + 4, :])
                dma_slice_u(1)
            dma_slice_u(u + 2)
            yield
            # q and k projections for this 512-token chunk; yield
            # mid-accumulation so the PE work spreads across the
            # interleaved attention pair steps
            for w_sb, dest in ((wq_sb, qT), (wk_sb, kTt)):
                pp = mm_ps.tile([128, 512], F32, tag="mm", name="pp")
                for kt in range(KT):
                    nc.tensor.matmul(pp[:], w_sb[:, kt * 128:(kt + 1) * 128],
                                     xTv[:, ch, kt, :],
                                     start=(kt == 0), stop=(kt == KT - 1))
                    if kt == 3:
                        yield
                nc.vector.tensor_copy(dest[:, ch * 512:(ch + 1) * 512], pp[:])
                yield
            # v for the same 4 token-tiles, token-major
            vv = vaug[:].rearrange("p (jt c) -> p jt c", c=130)
            vp = mm_ps.tile([128, 512], F32, tag="mm", name="vp")
            for j in range(4):
                for kt in range(KT):
                    nc.tensor.matmul(
                        vp[:, j * 128:(j + 1) * 128],
                        xTv[:, ch, kt, j * 128:(j + 1) * 128],
                        wv_sb[:, kt * 128:(kt + 1) * 128],
                        start=(kt == 0), stop=(kt == KT - 1))
                if j == 1:
                    yield
            src = vp[:].rearrange("p (j c) -> p j c", j=4)
            nc.vector.tensor_copy(vv[:, 4 * ch:4 * ch + 4, 0:64], src[:, :, 0:64])
            nc.vector.tensor_copy(vv[:, 4 * ch:4 * ch + 4, 65:129], src[:, :, 64:128])
            # ones columns via DVE (gpsimd's in-order queue is slow on
            # semaphore ops and would delay the normalize broadcast)
            nc.vector.tensor_copy(vv[:, 4 * ch:4 * ch + 4, 64:65],
                                  onesv[:, 4 * ch:4 * ch + 4, :])
            nc.vector.tensor_copy(vv[:, 4 * ch:4 * ch + 4, 129:130],
                                  onesv[:, 4 * ch:4 * ch + 4, :])
            yield

        def att_chunk_steps(b, c, hold=4):
            """Attention chunk c of batch b + deferred output projections."""
            t0 = b * T
            qT, kTt, vaug, outT = state[b]
            njt = 4 * (c + 1)
            avp = {h: av_ps_p.tile([65, 512], F32, tag="av", name=f"avp{h}")
                   for h in (0, 1)}

            def emit_av(pend):
                jt, off, at = pend
                for h in (0, 1):
                    nc.tensor.matmul(
                        avp[h][:, off:512],
                        vaug[:, jt * 130 + 65 * h: jt * 130 + 65 * h + 65],
                        at[:, h * 512 + off: (h + 1) * 512],
                        start=(jt == 0), stop=(jt == njt - 1))

            # one [128,1024] dots tile PER J-TILE from a double-buffered
            # pool: scores(jt+1) write the other buffer while exp(jt) reads,
            # so the PE NEVER waits on ACT for scores (no psum WAR).  The AV
            # matmuls lag 2 j-tiles so their exp is long done when the
            # in-order PE reaches them.
            pends = []
            for jt in range(njt):
                off = max(512 * c, jt * 128) - 512 * c
                dps = dots_ps.tile([128, 1024], F32, tag="dots", name="dp")
                # h0/h1 hit disjoint PE row groups (rows 0-63 / 64-127) and
                # run concurrently
                for h in (0, 1):
                    nc.tensor.matmul(
                        dps[:, h * 512 + off: (h + 1) * 512],
                        kTt[64 * h:64 * h + 64, jt * 128:(jt + 1) * 128],
                        qT[64 * h:64 * h + 64, 512 * c + off:512 * (c + 1)],
                        start=True, stop=True)
                at = attnT_p.tile([128, 1024], BF16, tag="at", name="at")
                # exp only the valid columns of both heads (strided AP)
                dv = dps[:].rearrange("p (h i) -> p h i", h=2)
                atv = at[:].rearrange("p (h i) -> p h i", h=2)
                nc.scalar.activation(atv[:, :, off:512], dv[:, :, off:512],
                                     mybir.ActivationFunctionType.Exp,
                                     bias=0.0, scale=float(SCALE))
                if jt >= 4 * c:  # zero invalid (j > i) entries of the diag tile
                    # one op for both heads via the strided [2, 128] AP
                    # (halves the gpsimd op+semaphore count on the chunk tail)
                    nc.gpsimd.affine_select(
                        out=atv[:, :, off: off + 128],
                        in_=atv[:, :, off: off + 128],
                        compare_op=mybir.AluOpType.is_ge, fill=0.0,
                        base=0, pattern=[[0, 2], [1, 128]], channel_multiplier=-1)
                if len(pends) >= 2:
                    emit_av(pends.pop(0))
                # oproj backlog as PE filler; hold back `hold` for later
                # (chunk tails + the ACT-bound final unit need PE filler)
                if len(deferred) > hold:
                    emit_oproj(*deferred.pop(0))
                pends.append((jt, off, at))
                yield
            # chunk tail: reserved oprojs interleave with the trailing AVs
            while pends:
                if deferred:
                    emit_oproj(*deferred.pop(0))
                emit_av(pends.pop(0))
            yield
            # normalize (fast recip keeps this short); the dependent oproj
            # is deferred into the next chunk's pairs.  NOTE: den must be
            # staged to a partition-0 tile - both the custom-DVE reciprocal
            # and any op pair with mismatched partition offsets misbehave.
            for h in (0, 1):
                den = recip_p.tile([1, 512], F32, tag="den", name="den")
                nc.vector.tensor_copy(den[:], avp[h][64:65, :])
                rc = recip_p.tile([1, 512], F32, tag="recip", name="rc")
                nc.vector.reciprocal_approx_fast(rc[:], den[:])
                rb = rbc_p.tile([64, 512], F32, tag="rbc", name="rb")
                nc.gpsimd.partition_broadcast(rb[:], rc[:])
                nc.vector.scalar_tensor_tensor(
                    outT[64 * h:64 * h + 64, c * 512:(c + 1) * 512],
                    avp[h][0:64, :], 1.0, rb[:],
                    op0=mybir.AluOpType.mult, op1=mybir.AluOpType.mult)
            deferred.extend((outT, t0, tt) for tt in range(4 * c, 4 * c + 4))
            yield

        def drive(gens):
            """Credit-weighted round-robin of (generator, weight) pairs: a
            generator advances ~weight steps per round, so both exhaust at
            the same time and the emission (= in-order engine queue order)
            keeps independent work spread between the dependency chains."""
            gens = [gw for gw in gens if gw is not None and gw[0] is not None]
            credit = [0.0] * len(gens)
            alive = [True] * len(gens)
            while any(alive):
                for i, (g, w) in enumerate(gens):
                    if not alive[i]:
                        continue
                    credit[i] += w
                    while credit[i] >= 1.0 and alive[i]:
                        credit[i] -= 1.0
                        try:
                            next(g)
                        except StopIteration:
                            alive[i] = False

        # oproj backlog floor per attention unit: steady 4 (defers each
        # chunk's oproj ~2 chunks), hoard through the penultimate units so
        # the ACT-bound final unit (no p12 partner) has PE filler, then
        # drain it there (hold 0)
        HOLDS = {(B - 1, NCHUNK - 3): 99, (B - 1, NCHUNK - 2): 99,
                 (B - 1, NCHUNK - 1): 0}
        P12_STEPS = 7.0
        units = [(b, ch) for b in range(B) for ch in range(NCHUNK)]
        prev = None
        for i, (b, ch) in enumerate(units):
            att = (att_chunk_steps(*prev, hold=HOLDS.get(prev, 4))
                   if prev is not None else None)
            att_steps = 4 * (prev[1] + 1) + 2 if prev is not None else 1
            drive([(att, 1.0) if att is not None else None,
                   (p12_chunk_steps(b, ch),
                    P12_STEPS / att_steps if att is not None else 4.0)])
            prev = (b, ch)
        drive([(att_chunk_steps(*prev, hold=HOLDS.get(prev, 4)), 1.0)])
        while deferred:
            emit_oproj(*deferred.pop(0))

    nc.compile()
    return nc


def kernel(x, Wq, Wkv, Wout, bout):
    """Full inputs -> full output. Shards across 8 NeuronCores internally."""
    global LAST_EXEC_NS, LAST_TRACE
    if "nc" not in _CACHED:
        _CACHED["nc"] = build_kernel()
    nc = _CACHED["nc"]

    hdt = ml_dtypes.bfloat16
    xf = np.asarray(x, dtype=np.float32).reshape(B, NCHUNK, 512, DIM)
    # [128, B, NCHUNK, KT, 512]: per-(chunk,partition) rows are 8KB
    # contiguous in DRAM so the per-chunk DMA moves full-size packets
    xT = np.ascontiguousarray(
        xf.transpose(3, 0, 1, 2)            # [DIM, B, NCHUNK, 512]
          .reshape(KT, 128, B, NCHUNK, 512)
          .transpose(1, 2, 3, 0, 4)).astype(hdt)
    xT = xT.reshape(128, B * NCHUNK * KT * 512)
    Wq = np.asarray(Wq, dtype=np.float32).astype(hdt)
    Wkv = np.asarray(Wkv, dtype=np.float32).astype(hdt)
    Wout = np.asarray(Wout, dtype=np.float32).astype(hdt)
    bout = np.asarray(bout, dtype=np.float32)

    def wlayout(w):  # [DIM, 128] -> [128, KT*128] SBUF layout for clean DMA
        return np.ascontiguousarray(
            w.reshape(KT, 128, 128).transpose(1, 0, 2).reshape(128, KT * 128))

    in_maps = []
    for c in range(8):
        s = slice(128 * c, 128 * (c + 1))
        in_maps.append({
            "xt": xT,
            "wq": wlayout(Wq[:, s]),
            "wk": wlayout(Wkv[:, :DIM][:, s]),
            "wv": wlayout(Wkv[:, DIM:][:, s]),
            "wo": np.ascontiguousarray(Wout[s, :]),
        })

    res = bass_utils.run_bass_kernel_spmd(nc, in_maps, core_ids=list(range(8)),
                                          trace=TRACE)
    if TRACE:
        LAST_EXEC_NS = res.exec_time_ns
        LAST_TRACE = res.instructions_and_trace
        globals()["LAST_PROFILE"] = getattr(res, "profile_json", None)
    acc = res.results[0]["out"].astype(np.float64)
    for c in range(1, 8):
        acc += res.results[c]["out"]
    out = (acc + bout.astype(np.float64)).astype(np.float32)
    return out.reshape(B, T, DIM)
